# revision 8
# baseline (speedup 1.0000x reference)
"""DeRA attention (2D-rope attention) Trainium2 kernel, 8-core head-parallel.

Sharding: 16 heads over 8 cores (2 heads/core). x is shipped
token-sharded (384 tokens/core, bf16, no host transpose) and
AllGathered on device, where PE transposes build the x^T tiles the
projections need. Each core computes q/k/v for its 2 heads, applies 2D
rope, dense attention over S=3072 interleaved per 1024-query block
with a token-major partial output projection (bias folded in via an
all-ones probability row and a bo/16 row in wo) and a chunked
ReduceScatter, so the cross-core reduction overlaps the next block's
attention. Each core returns only its 3x128-token slice of the final
output as bf16; the host scatters the 24 blocks into the fp32 result.

Layout tricks (all host-side prep is pure casting / block reshapes):
 - wq/wk columns are permuted per head to [id-evens, id-odds, rot-evens,
   rot-odds] so rope = elementwise mul with baked cos/sin tables plus a
   partition-block swap (done with 2 SBUF->SBUF DMAs).
 - scores are computed transposed (k on partitions) so the PV matmul needs
   no transpose of the probabilities; an all-ones column appended to V
   yields the softmax denominator inside the same PSUM accumulation.
"""

import sys

if "/opt/trn_rl_repo" not in sys.path:
    sys.path.insert(0, "/opt/trn_rl_repo")

import numpy as np
import ml_dtypes

BF16N = ml_dtypes.bfloat16

# Problem config (hardcoded per spec)
S = 3072
DIM = 1536
NH = 16
HD = 96
NCORES = 8
GH, GW = 48, 64
KC = DIM // 128          # 12 contraction chunks of 128
NQ = S // 512            # 6
NKC = S // 128           # 24 key chunks
QB = 1024                # q block (columns per attention round)
NQB = S // QB            # 3
XSH = S // NCORES        # 384 tokens shipped per core
OSH = S // NCORES        # 384 output tokens returned per core
SCALE = 1.0 / float(np.sqrt(HD))

# per-head column permutation: [id evens, id odds, rot evens, rot odds]
_PERM = np.asarray(
    [2 * j for j in range(16)]
    + [2 * j + 1 for j in range(16)]
    + [2 * j for j in range(16, 48)]
    + [2 * j + 1 for j in range(16, 48)]
)
# global column permutation for wq/wk: per-head perm applied to all 16 heads
_PERM_G = np.concatenate([h * HD + _PERM for h in range(NH)])

_STATE = {}


def _build_nc():
    import concourse.bass as bass  # noqa: F401
    import concourse.tile as tile
    from concourse import bacc, mybir
    from contextlib import ExitStack

    BF16 = mybir.dt.bfloat16
    F32 = mybir.dt.float32
    AF = mybir.ActivationFunctionType
    OP = mybir.AluOpType

    nc = bacc.Bacc("TRN2", target_bir_lowering=False, debug=False,
                   num_devices=NCORES)

    xsd = nc.dram_tensor("x_s", [XSH, DIM], BF16, kind="ExternalInput")
    wqd = nc.dram_tensor("wq_n", [DIM, 192], BF16, kind="ExternalInput")
    wkd = nc.dram_tensor("wk_n", [DIM, 192], BF16, kind="ExternalInput")
    wvd = nc.dram_tensor("wv_n", [DIM, 192], BF16, kind="ExternalInput")
    wod = nc.dram_tensor("wo_t", [97, 2 * DIM], BF16, kind="ExternalInput")
    bqd = nc.dram_tensor("bq_t", [96, 2], F32, kind="ExternalInput")
    bkd = nc.dram_tensor("bk_t", [96, 2], F32, kind="ExternalInput")
    bvd = nc.dram_tensor("bv_t", [96, 2], F32, kind="ExternalInput")
    cosd = nc.dram_tensor("cos_f", [64, S], BF16, kind="ExternalInput")
    sind = nc.dram_tensor("sin_f", [64, S], BF16, kind="ExternalInput")
    outd = nc.dram_tensor("outS", [NQB * 128, DIM], BF16,
                          kind="ExternalOutput")

    groups = [list(range(NCORES))]

    with tile.TileContext(nc) as tc:
        with ExitStack() as ctx:
            dram = ctx.enter_context(
                tc.tile_pool(name="dram", bufs=1, space="DRAM"))
            xin_b = dram.tile([XSH, DIM], BF16, name="xin_b")
            xg = dram.tile([S, DIM], BF16, name="xg")
            opart = dram.tile([S, DIM], F32, name="opart")
            oscat = dram.tile([NQB * 128, DIM], F32, name="oscat")

            # token slice -> bounce -> AllGather to full token-major x
            nc.gpsimd.dma_start(xin_b[:], xsd.ap())
            nc.gpsimd.collective_compute(
                "AllGather", mybir.AluOpType.bypass,
                replica_groups=groups,
                ins=[xin_b.opt()], outs=[xg.opt()])

            const = ctx.enter_context(tc.tile_pool(name="const", bufs=1))
            # compact [64, S] tables land on rows 32:96 so every rope
            # tensor_tensor op sees equal base partitions on both inputs
            cos_sb = const.tile([96, S], BF16, name="cos_sb")
            nc.sync.dma_start(out=cos_sb[32:96, :], in_=cosd.ap())
            sin_sb = const.tile([96, S], BF16, name="sin_sb")
            nc.sync.dma_start(out=sin_sb[32:96, :], in_=sind.ap())
            bq_sb = const.tile([96, 2], F32, name="bq_sb")
            nc.sync.dma_start(out=bq_sb[:], in_=bqd.ap())
            bk_sb = const.tile([96, 2], F32, name="bk_sb")
            nc.sync.dma_start(out=bk_sb[:], in_=bkd.ap())
            bv_sb = const.tile([96, 2], F32, name="bv_sb")
            nc.sync.dma_start(out=bv_sb[:], in_=bvd.ap())
            wo_sb = const.tile([97, 2 * DIM], BF16, name="wo_sb")
            nc.sync.dma_start(out=wo_sb[:], in_=wod.ap())
            from concourse.masks import make_identity
            ident = const.tile([128, 128], BF16, name="ident")
            make_identity(nc, ident[:])

            # tensors that persist from projection phase into attention
            mid = ctx.enter_context(tc.tile_pool(name="mid", bufs=1))
            v1 = {}
            attn = {}
            rope_out = {}
            for h in (0, 1):
                v1[h] = mid.tile([128, NKC * 128], BF16, name=f"v1_{h}",
                                 tag=f"v1_{h}")
                attn[h] = mid.tile([97, S], BF16, name=f"attn_{h}",
                                   tag=f"attn_{h}")
                # ones row: contributes wo's bias row once per (core, head)
                nc.gpsimd.memset(attn[h][96:97, :], 1.0)
                for t in ("q", "k"):
                    rope_out[(t, h)] = mid.tile([96, S], BF16,
                                                name=f"r_{t}{h}",
                                                tag=f"r_{t}{h}")

            # ---- phase 1: x^T tiles + projections + rope + v layout ----
            with tc.tile_pool(name="p3", bufs=1) as p3, \
                 tc.tile_pool(name="p3ps", bufs=8, space="PSUM") as p3ps:
                # weight tiling happens on-device: 12 row-block DMAs turn the
                # natural [1536, 192] slice into the [128, KC*192] SBUF layout
                wq_sb = p3.tile([128, KC * 192], BF16, name="wq_sb")
                wk_sb = p3.tile([128, KC * 192], BF16, name="wk_sb")
                wv_sb = p3.tile([128, KC * 192], BF16, name="wv_sb")
                for (wsb, wd) in ((wq_sb, wqd), (wk_sb, wkd), (wv_sb, wvd)):
                    for kc in range(KC):
                        nc.sync.dma_start(
                            out=wsb[:, kc * 192:(kc + 1) * 192],
                            in_=wd.ap()[kc * 128:(kc + 1) * 128, :])

                # x^T tiles via PE transposes of the gathered token-major x
                xt = []
                for kc in range(KC):
                    t_ = p3.tile([128, S], BF16, name=f"xt{kc}", tag="xt",
                                 bufs=KC)
                    xt.append(t_)
                for tg in range(6):          # groups of 4 token-tiles
                    xtoks = []
                    for j in range(4):
                        tk = tg * 4 + j
                        xt_j = p3.tile([128, DIM], BF16, name=f"xtok{tk}",
                                       tag="xtok", bufs=5)
                        nc.sync.dma_start(
                            out=xt_j[:],
                            in_=xg[tk * 128:(tk + 1) * 128, :])
                        xtoks.append(xt_j)
                    for kc in range(KC):
                        tp = p3ps.tile([128, 512], BF16,
                                       name=f"tp{tg}_{kc}", tag="tp", bufs=1)
                        for j in range(4):
                            nc.tensor.transpose(
                                tp[:, j * 128:(j + 1) * 128],
                                xtoks[j][:, kc * 128:(kc + 1) * 128],
                                ident[:])
                        if kc % 2:
                            nc.scalar.activation(
                                xt[kc][:, tg * 512:(tg + 1) * 512], tp[:],
                                AF.Copy)
                        else:
                            nc.vector.tensor_copy(
                                xt[kc][:, tg * 512:(tg + 1) * 512], tp[:])

                def project(wsb, bsb, h, dest):
                    ps = [p3ps.tile([96, 512], F32, name=f"ps{n}", tag="proj",
                                    bufs=6)
                          for n in range(NQ)]
                    for kc in range(KC):
                        lhs = wsb[:, kc * 192 + h * 96: kc * 192 + (h + 1) * 96]
                        for n in range(NQ):
                            nc.tensor.matmul(
                                ps[n][:], lhs,
                                xt[kc][:, n * 512:(n + 1) * 512],
                                start=(kc == 0), stop=(kc == KC - 1))
                    for n in range(NQ):
                        nc.vector.tensor_scalar_add(
                            dest[:, n * 512:(n + 1) * 512], ps[n][:],
                            bsb[:, h:h + 1])

                for h in (0, 1):
                    for (t, wsb, bsb) in (("q", wq_sb, bq_sb),
                                          ("k", wk_sb, bk_sb)):
                        pre = p3.tile([96, S], BF16, name=f"pre_{t}{h}",
                                      tag="pre", bufs=2)
                        project(wsb, bsb, h, pre)
                        # rope rows 0:32 are identity; rows 32:96 rotate by
                        # the compact [64, S] tables:
                        #   dst[32:96] = pre[32:96]*cos + swap(pre)[32:96]*sin
                        ro = rope_out[(t, h)]
                        nc.vector.tensor_copy(ro[0:32, :], pre[0:32, :])
                        sw = p3.tile([96, S], BF16, name=f"sw_{t}{h}",
                                     tag="sw", bufs=1)
                        nc.sync.dma_start(out=sw[32:64, :], in_=pre[64:96, :])
                        nc.sync.dma_start(out=sw[64:96, :], in_=pre[32:64, :])
                        t1 = p3.tile([96, S], BF16, name=f"t1_{t}{h}",
                                     tag="t1", bufs=1)
                        t2 = p3.tile([96, S], BF16, name=f"t2_{t}{h}",
                                     tag="t2", bufs=1)
                        for r0, r1 in ((32, 64), (64, 96)):
                            nc.vector.tensor_tensor(
                                t1[r0:r1, :], pre[r0:r1, :],
                                cos_sb[r0:r1, :], OP.mult)
                            nc.vector.tensor_tensor(
                                t2[r0:r1, :], sw[r0:r1, :],
                                sin_sb[r0:r1, :], OP.mult)
                            nc.vector.tensor_tensor(
                                ro[r0:r1, :], t1[r0:r1, :],
                                t2[r0:r1, :], OP.add)
                    vt_h = p3.tile([96, S], BF16, name=f"vt{h}", tag="vt",
                                   bufs=1)
                    project(wv_sb, bv_sb, h, vt_h)
                    for kc2 in range(NKC):
                        tr = p3ps.tile([128, 96], BF16,
                                       name=f"tr{h}_{kc2}", tag="tr",
                                       bufs=1)
                        nc.tensor.transpose(
                            tr[:], vt_h[:, kc2 * 128:(kc2 + 1) * 128],
                            ident[0:96, 0:96])
                        nc.vector.tensor_copy(
                            v1[h][:, kc2 * 128: kc2 * 128 + 96], tr[:])
                    ones_ap = v1[h].rearrange("p (k c) -> p k c", c=128)
                    nc.gpsimd.memset(ones_ap[:, :, 96:97], 1.0)

            # ---- phase 2+3: attention, o-projection and reduce, per
            # 1024-query block (both heads), so the ReduceScatter of block
            # qb overlaps the attention of block qb+1 ----
            with tc.tile_pool(name="p5", bufs=1) as p5, \
                 tc.tile_pool(name="p5ps", bufs=2, space="PSUM") as ps_s, \
                 tc.tile_pool(name="pvps", bufs=1, space="PSUM") as ps_pv, \
                 tc.tile_pool(name="p6ps", bufs=2, space="PSUM") as p6ps:
                for qb in range(NQB):
                    q0 = qb * QB
                    for h in (0, 1):
                        qT = rope_out[("q", h)]
                        kT = rope_out[("k", h)]
                        pv0 = ps_pv.tile([97, 512], F32, name=f"pv0_{h}{qb}",
                                         tag="pv0", bufs=1)
                        pv1 = ps_pv.tile([97, 512], F32, name=f"pv1_{h}{qb}",
                                         tag="pv1", bufs=1)
                        for kc2 in range(NKC):
                            sps = ps_s.tile([128, 1024], F32,
                                            name=f"s_{h}{qb}_{kc2}", tag="s",
                                            bufs=2)
                            lhs_k = kT[:, kc2 * 128:(kc2 + 1) * 128]
                            nc.tensor.matmul(sps[:, 0:512], lhs_k,
                                             qT[:, q0: q0 + 512])
                            nc.tensor.matmul(sps[:, 512:1024], lhs_k,
                                             qT[:, q0 + 512: q0 + 1024])
                            pt = p5.tile([128, 1024], BF16,
                                         name=f"pt_{h}{qb}_{kc2}", tag="pt",
                                         bufs=3)
                            nc.scalar.activation(pt[:], sps[:], AF.Exp,
                                                 scale=SCALE)
                            lhs_v = v1[h][:, kc2 * 128: kc2 * 128 + 97]
                            nc.tensor.matmul(pv0[:], lhs_v, pt[:, 0:512],
                                             start=(kc2 == 0),
                                             stop=(kc2 == NKC - 1))
                            nc.tensor.matmul(pv1[:], lhs_v, pt[:, 512:1024],
                                             start=(kc2 == 0),
                                             stop=(kc2 == NKC - 1))
                        for j, pv in enumerate((pv0, pv1)):
                            den = p5.tile([97, 512], F32,
                                          name=f"den{h}{qb}{j}", tag="den",
                                          bufs=2)
                            nc.vector.tensor_copy(den[96:97, :], pv[96:97, :])
                            dn0 = p5.tile([1, 512], F32,
                                          name=f"dn0{h}{qb}{j}", tag="dn0",
                                          bufs=2)
                            nc.sync.dma_start(out=dn0[:], in_=den[96:97, :])
                            rec = p5.tile([1, 512], F32,
                                          name=f"rec{h}{qb}{j}", tag="rec",
                                          bufs=2)
                            nc.vector.reciprocal(rec[:], dn0[:])
                            rb = p5.tile([96, 512], F32, name=f"rb{h}{qb}{j}",
                                         tag="rb", bufs=2)
                            nc.gpsimd.partition_broadcast(rb[:], rec[:])
                            nc.vector.tensor_tensor(
                                attn[h][0:96,
                                        q0 + j * 512: q0 + (j + 1) * 512],
                                pv[0:96, :], rb[:], OP.mult)

                    # token-major o-projection for this query block
                    for sb in range(qb * 8, (qb + 1) * 8):
                        for db in range(DIM // 512):
                            po = p6ps.tile([128, 512], F32,
                                           name=f"po{sb}_{db}", tag="po",
                                           bufs=2)
                            nc.tensor.matmul(
                                po[:], attn[0][:, sb * 128:(sb + 1) * 128],
                                wo_sb[:, db * 512:(db + 1) * 512],
                                start=True, stop=False)
                            nc.tensor.matmul(
                                po[:],
                                attn[1][:, sb * 128:(sb + 1) * 128],
                                wo_sb[:, DIM + db * 512: DIM + (db + 1) * 512],
                                start=False, stop=True)
                            ot = p5.tile([128, 512], F32, name=f"ot{sb}_{db}",
                                         tag="ot", bufs=6)
                            if db % 2:
                                nc.scalar.activation(ot[:], po[:], AF.Copy)
                            else:
                                nc.vector.tensor_copy(ot[:], po[:])
                            nc.sync.dma_start(
                                out=opart[sb * 128:(sb + 1) * 128,
                                          db * 512:(db + 1) * 512],
                                in_=ot[:])

                    # cross-core sum of this block; core c keeps 128 tokens
                    nc.gpsimd.collective_compute(
                        "ReduceScatter", mybir.AluOpType.add,
                        replica_groups=groups,
                        ins=[opart[q0:(qb + 1) * QB, :].opt()],
                        outs=[oscat[qb * 128:(qb + 1) * 128, :].opt()])
                    # fp32 -> bf16 for the ride home
                    oc_f = p5.tile([128, DIM], F32, name=f"ocf{qb}",
                                   tag="ocf", bufs=2)
                    nc.sync.dma_start(out=oc_f[:],
                                      in_=oscat[qb * 128:(qb + 1) * 128, :])
                    oc_b = p5.tile([128, DIM], BF16, name=f"ocb{qb}",
                                   tag="ocb", bufs=2)
                    nc.vector.tensor_copy(oc_b[:], oc_f[:])
                    nc.sync.dma_start(
                        out=outd.ap()[qb * 128:(qb + 1) * 128, :],
                        in_=oc_b[:])

    nc.compile()
    return nc


def _get_nc():
    if "nc" not in _STATE:
        _STATE["nc"] = _build_nc()
    return _STATE["nc"]


def _get_runner():
    """Cached jitted shard_map over the 8 cores (replaces
    run_bass_kernel_spmd's per-call jit rebuild)."""
    if "runner" in _STATE:
        return _STATE["runner"]
    nc = _get_nc()
    import jax
    from jax.sharding import Mesh, PartitionSpec
    try:
        from jax.experimental.shard_map import shard_map
    except ImportError:                      # newer jax
        from jax.shard_map import shard_map  # type: ignore
    from concourse import bass2jax, mybir

    bass2jax.install_neuronx_cc_hook()
    partition_name = (nc.partition_id_tensor.name
                      if nc.partition_id_tensor else None)
    in_names = []
    out_names = []
    out_avals = []
    for alloc in nc.m.functions[0].allocations:
        if not isinstance(alloc, mybir.MemoryLocationSet):
            continue
        name = alloc.memorylocations[0].name
        if alloc.kind == "ExternalInput":
            if name != partition_name:
                in_names.append(name)
        elif alloc.kind == "ExternalOutput":
            out_names.append(name)
            out_avals.append(jax.core.ShapedArray(
                tuple(alloc.tensor_shape), mybir.dt.np(alloc.dtype)))
    n_params = len(in_names)
    n_outs = len(out_avals)
    all_names = list(in_names) + list(out_names)
    if partition_name is not None:
        all_names.append(partition_name)
    donate = tuple(range(n_params, n_params + n_outs))

    def _body(*args):
        operands = list(args)
        if partition_name is not None:
            operands.append(bass2jax.partition_id_tensor())
        outs = bass2jax._bass_exec_p.bind(
            *operands,
            out_avals=tuple(out_avals),
            in_names=tuple(all_names),
            out_names=tuple(out_names),
            lowering_input_output_aliases=(),
            sim_require_finite=True,
            sim_require_nnan=True,
            nc=nc,
        )
        return tuple(outs)

    devices = jax.devices()[:NCORES]
    mesh = Mesh(np.asarray(devices), ("core",))
    in_specs = (PartitionSpec("core"),) * (n_params + n_outs)
    out_specs = (PartitionSpec("core"),) * n_outs
    fn = jax.jit(
        shard_map(_body, mesh=mesh, in_specs=in_specs,
                  out_specs=out_specs, check_rep=False),
        donate_argnums=donate, keep_unused=True)
    zero_shapes = [(NCORES * a.shape[0], *a.shape[1:]) for a in out_avals]
    zero_dtypes = [a.dtype for a in out_avals]
    _STATE["runner"] = (fn, in_names, out_names, out_avals,
                        zero_shapes, zero_dtypes)
    return _STATE["runner"]


def make_global_inputs(x, wq, bq, wk, bk, wv, bv, wo, bo, freqs_cos,
                       freqs_sin, h, w):
    """Build the already-concatenated [NCORES*d0, ...] global input arrays
    the sharded runner hands one slice of to each core."""
    assert int(h) == GH and int(w) == GW
    x = np.asarray(x, np.float32)
    wq = np.asarray(wq, np.float32)
    wk = np.asarray(wk, np.float32)
    wv = np.asarray(wv, np.float32)
    wo = np.asarray(wo, np.float32)
    bq = np.asarray(bq, np.float32)
    bk = np.asarray(bk, np.float32)
    bv = np.asarray(bv, np.float32)
    bo = np.asarray(bo, np.float32)
    fc = np.asarray(freqs_cos, np.float32)
    fs = np.asarray(freqs_sin, np.float32)

    # rope tables in the permuted row basis, compact rows 32:96
    tpos = np.arange(S)
    gh = tpos // GW
    gw = tpos % GW
    c32 = np.empty((32, S), np.float32)
    s32 = np.empty((32, S), np.float32)
    c32[0:16] = fc[gh, 16:32].T
    c32[16:32] = fc[gw, 32:48].T
    s32[0:16] = fs[gh, 16:32].T
    s32[16:32] = fs[gw, 32:48].T
    cosF = np.empty((64, S), np.float32)
    cosF[0:32] = c32
    cosF[32:64] = c32
    sinF = np.empty((64, S), np.float32)
    sinF[0:32] = -s32
    sinF[32:64] = s32
    cos_g = np.broadcast_to(cosF.astype(BF16N), (NCORES, 64, S))
    sin_g = np.broadcast_to(sinF.astype(BF16N), (NCORES, 64, S))

    def shard_cols(wb):
        # [1536, 1536] -> [8*1536, 192]: core c gets columns 192c..192(c+1)
        return np.ascontiguousarray(
            wb.reshape(DIM, NCORES, 192).transpose(1, 0, 2)
        ).reshape(NCORES * DIM, 192)

    wq_g = shard_cols(wq.astype(BF16N)[:, _PERM_G])
    wk_g = shard_cols(wk.astype(BF16N)[:, _PERM_G])
    wv_g = shard_cols(wv.astype(BF16N))

    wo_bf = wo.astype(BF16N)
    bo16 = (bo / 16.0).astype(BF16N)
    wo_g = np.empty((NCORES, 97, 2 * DIM), BF16N)
    for c in range(NCORES):
        wo_g[c, 0:96, 0:DIM] = wo_bf[c * 192: c * 192 + 96]
        wo_g[c, 0:96, DIM:] = wo_bf[c * 192 + 96: c * 192 + 192]
        wo_g[c, 96, 0:DIM] = bo16
        wo_g[c, 96, DIM:] = bo16

    bq_p = bq.reshape(NH, HD)[:, _PERM]     # [16, 96]
    bk_p = bk.reshape(NH, HD)[:, _PERM]
    bv_r = bv.reshape(NH, HD)
    bq_g = bq_p.reshape(NCORES, 2, 96).transpose(0, 2, 1)   # [8, 96, 2]
    bk_g = bk_p.reshape(NCORES, 2, 96).transpose(0, 2, 1)
    bv_g = bv_r.reshape(NCORES, 2, 96).transpose(0, 2, 1)

    return {
        "x_s": np.ascontiguousarray(x[0]).astype(BF16N),   # [3072, 1536]
        "wq_n": wq_g, "wk_n": wk_g, "wv_n": wv_g,
        "wo_t": np.ascontiguousarray(wo_g).reshape(NCORES * 97, 2 * DIM),
        "bq_t": np.ascontiguousarray(bq_g, dtype=np.float32
                                     ).reshape(NCORES * 96, 2),
        "bk_t": np.ascontiguousarray(bk_g, dtype=np.float32
                                     ).reshape(NCORES * 96, 2),
        "bv_t": np.ascontiguousarray(bv_g, dtype=np.float32
                                     ).reshape(NCORES * 96, 2),
        "cos_f": np.ascontiguousarray(cos_g).reshape(NCORES * 64, S),
        "sin_f": np.ascontiguousarray(sin_g).reshape(NCORES * 64, S),
    }


def kernel(x, wq, bq, wk, bk, wv, bv, wo, bo, freqs_cos, freqs_sin, h, w):
    fn, in_names, out_names, out_avals, zero_shapes, zero_dtypes = \
        _get_runner()
    gin = make_global_inputs(x, wq, bq, wk, bk, wv, bv, wo, bo,
                             freqs_cos, freqs_sin, h, w)
    args = [gin[nm] for nm in in_names]
    args += [np.zeros(s, d) for s, d in zip(zero_shapes, zero_dtypes)]
    out_arrs = fn(*args)
    og = np.asarray(out_arrs[out_names.index("outS")])   # [8*384, 1536] bf16
    og = og.reshape(NCORES, NQB, 128, DIM)
    out = np.empty((1, S, DIM), np.float32)
    for qb in range(NQB):
        for c in range(NCORES):
            t0 = qb * QB + c * 128
            out[0, t0:t0 + 128] = og[c, qb]
    return out


# revision 11
# speedup vs baseline: 1.0613x; 1.0613x over previous
"""DeRA attention (2D-rope attention) Trainium2 kernel, 8-core head-parallel.

Sharding: 16 heads over 8 cores (2 heads/core). x is shipped
token-sharded (384 tokens/core, bf16). Each core PE-transposes its own
slice (36 small transposes, before any communication), then a
feature-chunked AllGather assembles x^T so projections can start
accumulating after the first chunk. Each core computes q/k/v for its 2
heads, applies 2D rope, dense attention over S=3072 per 1024-query
block (both heads), a token-major partial output projection (bias
folded in via an all-ones probability row and a bo/16 row in wo), and
a chunked ReduceScatter so the cross-core reduction overlaps the next
block's attention. The softmax normalization uses a DRAM-bounce
partition broadcast + reciprocal on the vector engine, keeping the
gpsimd queue free for the collectives. Each core returns its 384
tokens of the final output as bf16; the host scatters 4x8 blocks into
the fp32 result.

Layout tricks (all host-side prep is pure casting / block reshapes):
 - wq/wk columns are permuted per head to [id-evens, id-odds, rot-evens,
   rot-odds] so rope = elementwise mul with baked cos/sin tables plus a
   partition-block swap (done with 2 SBUF->SBUF DMAs).
 - scores are computed transposed (k on partitions) so the PV matmul needs
   no transpose of the probabilities; an all-ones column appended to V
   yields the softmax denominator inside the same PSUM accumulation.
"""

import sys

if "/opt/trn_rl_repo" not in sys.path:
    sys.path.insert(0, "/opt/trn_rl_repo")

import numpy as np
import ml_dtypes

BF16N = ml_dtypes.bfloat16

# Problem config (hardcoded per spec)
S = 3072
DIM = 1536
NH = 16
HD = 96
NCORES = 8
GH, GW = 48, 64
KC = DIM // 128          # 12 contraction chunks of 128
NKC = S // 128           # 24 key chunks
QB = 1024                # q block (columns per attention round)
NQB = S // QB            # 3
XSH = S // NCORES        # 384 tokens shipped per core
SCALE = 1.0 / float(np.sqrt(HD))
# ReduceScatter chunks: (sb0, sb1, oscat row offset); sb = 128-token blocks
RS_CHUNKS = {0: [(0, 8, 0)], 1: [(8, 16, 128)],
             2: [(16, 20, 256), (20, 24, 320)]}

# per-head column permutation: [id evens, id odds, rot evens, rot odds]
_PERM = np.asarray(
    [2 * j for j in range(16)]
    + [2 * j + 1 for j in range(16)]
    + [2 * j for j in range(16, 48)]
    + [2 * j + 1 for j in range(16, 48)]
)
# global column permutation for wq/wk: per-head perm applied to all 16 heads
_PERM_G = np.concatenate([h * HD + _PERM for h in range(NH)])

_STATE = {}


def _build_nc():
    import concourse.bass as bass  # noqa: F401
    import concourse.tile as tile
    from concourse import bacc, mybir
    from contextlib import ExitStack

    BF16 = mybir.dt.bfloat16
    F32 = mybir.dt.float32
    AF = mybir.ActivationFunctionType
    OP = mybir.AluOpType

    nc = bacc.Bacc("TRN2", target_bir_lowering=False, debug=False,
                   num_devices=NCORES)

    xsd = nc.dram_tensor("x_s", [XSH, DIM], BF16, kind="ExternalInput")
    wqd = nc.dram_tensor("wq_n", [DIM, 192], BF16, kind="ExternalInput")
    wkd = nc.dram_tensor("wk_n", [DIM, 192], BF16, kind="ExternalInput")
    wvd = nc.dram_tensor("wv_n", [DIM, 192], BF16, kind="ExternalInput")
    wod = nc.dram_tensor("wo_t", [97, 2 * DIM], BF16, kind="ExternalInput")
    bqd = nc.dram_tensor("bq_t", [96, 2], F32, kind="ExternalInput")
    bkd = nc.dram_tensor("bk_t", [96, 2], F32, kind="ExternalInput")
    bvd = nc.dram_tensor("bv_t", [96, 2], F32, kind="ExternalInput")
    cosd = nc.dram_tensor("cos_f", [64, S], BF16, kind="ExternalInput")
    sind = nc.dram_tensor("sin_f", [64, S], BF16, kind="ExternalInput")
    outd = nc.dram_tensor("outS", [XSH, DIM], BF16, kind="ExternalOutput")

    groups = [list(range(NCORES))]

    with tile.TileContext(nc) as tc:
        with ExitStack() as ctx:
            dram = ctx.enter_context(
                tc.tile_pool(name="dram", bufs=1, space="DRAM"))
            xts_d = dram.tile([DIM, XSH], BF16, name="xts_d")
            xgc = [dram.tile([512 * NCORES, XSH], BF16, name=f"xgc{f}")
                   for f in range(3)]
            opart = dram.tile([S, DIM], F32, name="opart")
            oscat = dram.tile([XSH, DIM], F32, name="oscat")

            const = ctx.enter_context(tc.tile_pool(name="const", bufs=1))
            # compact [64, S] tables land on rows 32:96 so every rope
            # tensor_tensor op sees equal base partitions on both inputs
            cos_sb = const.tile([96, S], BF16, name="cos_sb")
            nc.sync.dma_start(out=cos_sb[32:96, :], in_=cosd.ap())
            sin_sb = const.tile([96, S], BF16, name="sin_sb")
            nc.sync.dma_start(out=sin_sb[32:96, :], in_=sind.ap())
            bq_sb = const.tile([96, 2], F32, name="bq_sb")
            nc.sync.dma_start(out=bq_sb[:], in_=bqd.ap())
            bk_sb = const.tile([96, 2], F32, name="bk_sb")
            nc.sync.dma_start(out=bk_sb[:], in_=bkd.ap())
            bv_sb = const.tile([96, 2], F32, name="bv_sb")
            nc.sync.dma_start(out=bv_sb[:], in_=bvd.ap())
            wo_sb = const.tile([97, 2 * DIM], BF16, name="wo_sb")
            nc.sync.dma_start(out=wo_sb[:], in_=wod.ap())
            from concourse.masks import make_identity
            ident = const.tile([128, 128], BF16, name="ident")
            make_identity(nc, ident[:])

            # tensors that persist from projection phase into attention
            mid = ctx.enter_context(tc.tile_pool(name="mid", bufs=1))
            v1 = {}
            attn = {}
            rope_out = {}
            for h in (0, 1):
                v1[h] = mid.tile([128, NKC * 128], BF16, name=f"v1_{h}",
                                 tag=f"v1_{h}")
                attn[h] = mid.tile([97, S], BF16, name=f"attn_{h}",
                                   tag=f"attn_{h}")
                # ones row: contributes wo's bias row once per (core, head)
                nc.gpsimd.memset(attn[h][96:97, :], 1.0)
                for t in ("q", "k"):
                    rope_out[(t, h)] = mid.tile([96, S], BF16,
                                                name=f"r_{t}{h}",
                                                tag=f"r_{t}{h}")

            # ---- phase 1: x^T assembly + projections + rope + v layout ----
            with tc.tile_pool(name="p3", bufs=1) as p3, \
                 tc.tile_pool(name="p3ps", bufs=8, space="PSUM") as p3ps:
                # transpose our own 384 tokens BEFORE the AllGather: 36 small
                # PE transposes, off the communication critical path
                xs_sb = []
                for j in range(3):
                    t_ = p3.tile([128, DIM], BF16, name=f"xs{j}", tag="xs",
                                 bufs=3)
                    nc.sync.dma_start(
                        out=t_[:], in_=xsd.ap()[j * 128:(j + 1) * 128, :])
                    xs_sb.append(t_)
                for fc in range(KC):
                    tpx = p3ps.tile([128, XSH], BF16, name=f"tpx{fc}",
                                    tag="tpx", bufs=1)
                    for j in range(3):
                        nc.tensor.transpose(
                            tpx[:, j * 128:(j + 1) * 128],
                            xs_sb[j][:, fc * 128:(fc + 1) * 128], ident[:])
                    xts = p3.tile([128, XSH], BF16, name=f"xts{fc}",
                                  tag="xts", bufs=2)
                    if fc % 2:
                        nc.scalar.activation(xts[:], tpx[:], AF.Copy)
                    else:
                        nc.vector.tensor_copy(xts[:], tpx[:])
                    nc.sync.dma_start(
                        out=xts_d[fc * 128:(fc + 1) * 128, :], in_=xts[:])

                # feature-chunked AllGather of x^T (3 x 512 feature rows)
                for f in range(3):
                    nc.gpsimd.collective_compute(
                        "AllGather", mybir.AluOpType.bypass,
                        replica_groups=groups,
                        ins=[xts_d[f * 512:(f + 1) * 512, :].opt()],
                        outs=[xgc[f].opt()])

                # weight tiling on-device: 12 row-block DMAs per weight
                wq_sb = p3.tile([128, KC * 192], BF16, name="wq_sb")
                wk_sb = p3.tile([128, KC * 192], BF16, name="wk_sb")
                wv_sb = p3.tile([128, KC * 192], BF16, name="wv_sb")
                for (wsb, wd) in ((wq_sb, wqd), (wk_sb, wkd), (wv_sb, wvd)):
                    for kc in range(KC):
                        nc.sync.dma_start(
                            out=wsb[:, kc * 192:(kc + 1) * 192],
                            in_=wd.ap()[kc * 128:(kc + 1) * 128, :])

                # reassemble x^T tiles: one strided DMA per kc pulls this
                # feature block's 128 rows x (8 cores x 384 tokens)
                xt = []
                for kc in range(KC):
                    t_ = p3.tile([128, S], BF16, name=f"xt{kc}", tag="xt",
                                 bufs=KC)
                    f, kcl = kc // 4, kc % 4
                    src = xgc[f].rearrange("(c k p) t -> k p c t",
                                           c=NCORES, k=4)[kcl]
                    nc.sync.dma_start(
                        out=t_[:].rearrange("p (c t) -> p c t", c=NCORES),
                        in_=src)
                    xt.append(t_)

                def project(wsb, bsb, h, dest):
                    ps = [p3ps.tile([96, 1024], F32, name=f"ps{n3}",
                                    tag="proj", bufs=3)
                          for n3 in range(3)]
                    for kc in range(KC):
                        lhs = wsb[:, kc * 192 + h * 96: kc * 192 + (h + 1) * 96]
                        for n3 in range(3):
                            for hb in (0, 1):   # one PSUM bank per matmul
                                nc.tensor.matmul(
                                    ps[n3][:, hb * 512:(hb + 1) * 512], lhs,
                                    xt[kc][:, n3 * 1024 + hb * 512:
                                           n3 * 1024 + (hb + 1) * 512],
                                    start=(kc == 0), stop=(kc == KC - 1))
                    for n3 in range(3):
                        nc.vector.tensor_scalar_add(
                            dest[:, n3 * 1024:(n3 + 1) * 1024], ps[n3][:],
                            bsb[:, h:h + 1])

                for h in (0, 1):
                    for (t, wsb, bsb) in (("q", wq_sb, bq_sb),
                                          ("k", wk_sb, bk_sb)):
                        pre = p3.tile([96, S], BF16, name=f"pre_{t}{h}",
                                      tag="pre", bufs=2)
                        project(wsb, bsb, h, pre)
                        # rope rows 0:32 are identity; rows 32:96 rotate by
                        # the tables: dst = pre*cos + swap(pre)*sin
                        ro = rope_out[(t, h)]
                        nc.vector.tensor_copy(ro[0:32, :], pre[0:32, :])
                        sw = p3.tile([96, S], BF16, name=f"sw_{t}{h}",
                                     tag="sw", bufs=1)
                        nc.sync.dma_start(out=sw[32:64, :], in_=pre[64:96, :])
                        nc.sync.dma_start(out=sw[64:96, :], in_=pre[32:64, :])
                        t1 = p3.tile([96, S], BF16, name=f"t1_{t}{h}",
                                     tag="t1", bufs=1)
                        t2 = p3.tile([96, S], BF16, name=f"t2_{t}{h}",
                                     tag="t2", bufs=1)
                        for r0, r1 in ((32, 64), (64, 96)):
                            nc.vector.tensor_tensor(
                                t1[r0:r1, :], pre[r0:r1, :],
                                cos_sb[r0:r1, :], OP.mult)
                            nc.vector.tensor_tensor(
                                t2[r0:r1, :], sw[r0:r1, :],
                                sin_sb[r0:r1, :], OP.mult)
                            nc.vector.tensor_tensor(
                                ro[r0:r1, :], t1[r0:r1, :],
                                t2[r0:r1, :], OP.add)
                    vt_h = p3.tile([96, S], BF16, name=f"vt{h}", tag="vt",
                                   bufs=1)
                    project(wv_sb, bv_sb, h, vt_h)
                    for kc2 in range(NKC):
                        tr = p3ps.tile([128, 96], BF16,
                                       name=f"tr{h}_{kc2}", tag="tr",
                                       bufs=1)
                        nc.tensor.transpose(
                            tr[:], vt_h[:, kc2 * 128:(kc2 + 1) * 128],
                            ident[0:96, 0:96])
                        nc.vector.tensor_copy(
                            v1[h][:, kc2 * 128: kc2 * 128 + 96], tr[:])
                    ones_ap = v1[h].rearrange("p (k c) -> p k c", c=128)
                    nc.gpsimd.memset(ones_ap[:, :, 96:97], 1.0)

            # ---- phase 2+3: attention, o-projection and reduce, per
            # 1024-query block (both heads); ReduceScatter chunks overlap
            # the next block's attention ----
            with tc.tile_pool(name="p5", bufs=1) as p5, \
                 tc.tile_pool(name="p5ps", bufs=2, space="PSUM") as ps_s, \
                 tc.tile_pool(name="pvps", bufs=1, space="PSUM") as ps_pv, \
                 tc.tile_pool(name="p6ps", bufs=2, space="PSUM") as p6ps:
                for qb in range(NQB):
                    q0 = qb * QB
                    for h in (0, 1):
                        qT = rope_out[("q", h)]
                        kT = rope_out[("k", h)]
                        pv = ps_pv.tile([97, 1024], F32, name=f"pv_{h}{qb}",
                                        tag="pv", bufs=1)
                        for kc2 in range(NKC):
                            sps = ps_s.tile([128, 1024], F32,
                                            name=f"s_{h}{qb}_{kc2}", tag="s",
                                            bufs=2)
                            lhs_k = kT[:, kc2 * 128:(kc2 + 1) * 128]
                            nc.tensor.matmul(sps[:, 0:512], lhs_k,
                                             qT[:, q0: q0 + 512])
                            nc.tensor.matmul(sps[:, 512:1024], lhs_k,
                                             qT[:, q0 + 512: q0 + 1024])
                            pt = p5.tile([128, 1024], BF16,
                                         name=f"pt_{h}{qb}_{kc2}", tag="pt",
                                         bufs=3)
                            nc.scalar.activation(pt[:], sps[:], AF.Exp,
                                                 scale=SCALE)
                            lhs_v = v1[h][:, kc2 * 128: kc2 * 128 + 97]
                            nc.tensor.matmul(pv[:, 0:512], lhs_v,
                                             pt[:, 0:512],
                                             start=(kc2 == 0),
                                             stop=(kc2 == NKC - 1))
                            nc.tensor.matmul(pv[:, 512:1024], lhs_v,
                                             pt[:, 512:1024],
                                             start=(kc2 == 0),
                                             stop=(kc2 == NKC - 1))
                        # normalization: DRAM-bounce broadcast of the
                        # denominator row, reciprocal on 96 lanes, multiply.
                        # No gpsimd -> collectives never block attention.
                        den = p5.tile([97, 1024], F32, name=f"den{h}{qb}",
                                      tag="den", bufs=2)
                        nc.vector.tensor_copy(den[96:97, :], pv[96:97, :])
                        dsc = dram.tile([1, 1024], F32, name=f"dsc{h}{qb}",
                                        tag="dsc", bufs=2)
                        nc.sync.dma_start(out=dsc[:], in_=den[96:97, :])
                        rb = p5.tile([96, 1024], F32, name=f"rb{h}{qb}",
                                     tag="rb", bufs=2)
                        nc.sync.dma_start(
                            out=rb[:], in_=dsc[:].broadcast_to((96, 1024)))
                        rbi = p5.tile([96, 1024], F32, name=f"rbi{h}{qb}",
                                      tag="rbi", bufs=2)
                        nc.vector.reciprocal(rbi[:], rb[:])
                        nc.vector.tensor_tensor(
                            attn[h][0:96, q0: q0 + 1024],
                            pv[0:96, :], rbi[:], OP.mult)

                    # token-major o-projection + chunked cross-core reduce
                    for (sb0, sb1, oroff) in RS_CHUNKS[qb]:
                        for sb in range(sb0, sb1):
                            for db in range(DIM // 512):
                                po = p6ps.tile([128, 512], F32,
                                               name=f"po{sb}_{db}", tag="po",
                                               bufs=2)
                                nc.tensor.matmul(
                                    po[:],
                                    attn[0][:, sb * 128:(sb + 1) * 128],
                                    wo_sb[:, db * 512:(db + 1) * 512],
                                    start=True, stop=False)
                                nc.tensor.matmul(
                                    po[:],
                                    attn[1][:, sb * 128:(sb + 1) * 128],
                                    wo_sb[:, DIM + db * 512:
                                          DIM + (db + 1) * 512],
                                    start=False, stop=True)
                                ot = p5.tile([128, 512], F32,
                                             name=f"ot{sb}_{db}", tag="ot",
                                             bufs=6)
                                if db % 2:
                                    nc.scalar.activation(ot[:], po[:],
                                                         AF.Copy)
                                else:
                                    nc.vector.tensor_copy(ot[:], po[:])
                                nc.sync.dma_start(
                                    out=opart[sb * 128:(sb + 1) * 128,
                                              db * 512:(db + 1) * 512],
                                    in_=ot[:])
                        rows = (sb1 - sb0) * 128 // NCORES
                        nc.gpsimd.collective_compute(
                            "ReduceScatter", mybir.AluOpType.add,
                            replica_groups=groups,
                            ins=[opart[sb0 * 128: sb1 * 128, :].opt()],
                            outs=[oscat[oroff: oroff + rows, :].opt()])
                        # fp32 -> bf16 for the ride home
                        oc_f = p5.tile([128, DIM], F32, name=f"ocf{sb0}",
                                       tag="ocf", bufs=2)
                        nc.sync.dma_start(
                            out=oc_f[0:rows, :],
                            in_=oscat[oroff: oroff + rows, :])
                        oc_b = p5.tile([128, DIM], BF16, name=f"ocb{sb0}",
                                       tag="ocb", bufs=2)
                        nc.vector.tensor_copy(oc_b[0:rows, :],
                                              oc_f[0:rows, :])
                        nc.sync.dma_start(
                            out=outd.ap()[oroff: oroff + rows, :],
                            in_=oc_b[0:rows, :])

    nc.compile()
    return nc


def _get_nc():
    if "nc" not in _STATE:
        _STATE["nc"] = _build_nc()
    return _STATE["nc"]


def _get_runner():
    """Cached jitted shard_map over the 8 cores (replaces
    run_bass_kernel_spmd's per-call jit rebuild)."""
    if "runner" in _STATE:
        return _STATE["runner"]
    nc = _get_nc()
    import jax
    from jax.sharding import Mesh, PartitionSpec
    try:
        from jax.experimental.shard_map import shard_map
    except ImportError:                      # newer jax
        from jax.shard_map import shard_map  # type: ignore
    from concourse import bass2jax, mybir

    bass2jax.install_neuronx_cc_hook()
    partition_name = (nc.partition_id_tensor.name
                      if nc.partition_id_tensor else None)
    in_names = []
    out_names = []
    out_avals = []
    for alloc in nc.m.functions[0].allocations:
        if not isinstance(alloc, mybir.MemoryLocationSet):
            continue
        name = alloc.memorylocations[0].name
        if alloc.kind == "ExternalInput":
            if name != partition_name:
                in_names.append(name)
        elif alloc.kind == "ExternalOutput":
            out_names.append(name)
            out_avals.append(jax.core.ShapedArray(
                tuple(alloc.tensor_shape), mybir.dt.np(alloc.dtype)))
    n_params = len(in_names)
    n_outs = len(out_avals)
    all_names = list(in_names) + list(out_names)
    if partition_name is not None:
        all_names.append(partition_name)
    donate = tuple(range(n_params, n_params + n_outs))

    def _body(*args):
        operands = list(args)
        if partition_name is not None:
            operands.append(bass2jax.partition_id_tensor())
        outs = bass2jax._bass_exec_p.bind(
            *operands,
            out_avals=tuple(out_avals),
            in_names=tuple(all_names),
            out_names=tuple(out_names),
            lowering_input_output_aliases=(),
            sim_require_finite=True,
            sim_require_nnan=True,
            nc=nc,
        )
        return tuple(outs)

    devices = jax.devices()[:NCORES]
    mesh = Mesh(np.asarray(devices), ("core",))
    in_specs = (PartitionSpec("core"),) * (n_params + n_outs)
    out_specs = (PartitionSpec("core"),) * n_outs
    fn = jax.jit(
        shard_map(_body, mesh=mesh, in_specs=in_specs,
                  out_specs=out_specs, check_rep=False),
        donate_argnums=donate, keep_unused=True)
    zero_shapes = [(NCORES * a.shape[0], *a.shape[1:]) for a in out_avals]
    zero_dtypes = [a.dtype for a in out_avals]
    _STATE["runner"] = (fn, in_names, out_names, out_avals,
                        zero_shapes, zero_dtypes)
    return _STATE["runner"]


def make_global_inputs(x, wq, bq, wk, bk, wv, bv, wo, bo, freqs_cos,
                       freqs_sin, h, w):
    """Build the already-concatenated [NCORES*d0, ...] global input arrays
    the sharded runner hands one slice of to each core."""
    assert int(h) == GH and int(w) == GW
    x = np.asarray(x, np.float32)
    wq = np.asarray(wq, np.float32)
    wk = np.asarray(wk, np.float32)
    wv = np.asarray(wv, np.float32)
    wo = np.asarray(wo, np.float32)
    bq = np.asarray(bq, np.float32)
    bk = np.asarray(bk, np.float32)
    bv = np.asarray(bv, np.float32)
    bo = np.asarray(bo, np.float32)
    fc = np.asarray(freqs_cos, np.float32)
    fs = np.asarray(freqs_sin, np.float32)

    # rope tables in the permuted row basis, compact rows 32:96
    tpos = np.arange(S)
    gh = tpos // GW
    gw = tpos % GW
    c32 = np.empty((32, S), np.float32)
    s32 = np.empty((32, S), np.float32)
    c32[0:16] = fc[gh, 16:32].T
    c32[16:32] = fc[gw, 32:48].T
    s32[0:16] = fs[gh, 16:32].T
    s32[16:32] = fs[gw, 32:48].T
    cosF = np.empty((64, S), np.float32)
    cosF[0:32] = c32
    cosF[32:64] = c32
    sinF = np.empty((64, S), np.float32)
    sinF[0:32] = -s32
    sinF[32:64] = s32
    cos_g = np.broadcast_to(cosF.astype(BF16N), (NCORES, 64, S))
    sin_g = np.broadcast_to(sinF.astype(BF16N), (NCORES, 64, S))

    def shard_cols(wb):
        # [1536, 1536] -> [8*1536, 192]: core c gets columns 192c..192(c+1)
        return np.ascontiguousarray(
            wb.reshape(DIM, NCORES, 192).transpose(1, 0, 2)
        ).reshape(NCORES * DIM, 192)

    wq_g = shard_cols(wq.astype(BF16N)[:, _PERM_G])
    wk_g = shard_cols(wk.astype(BF16N)[:, _PERM_G])
    wv_g = shard_cols(wv.astype(BF16N))

    wo_bf = wo.astype(BF16N)
    bo16 = (bo / 16.0).astype(BF16N)
    wo_g = np.empty((NCORES, 97, 2 * DIM), BF16N)
    for c in range(NCORES):
        wo_g[c, 0:96, 0:DIM] = wo_bf[c * 192: c * 192 + 96]
        wo_g[c, 0:96, DIM:] = wo_bf[c * 192 + 96: c * 192 + 192]
        wo_g[c, 96, 0:DIM] = bo16
        wo_g[c, 96, DIM:] = bo16

    bq_p = bq.reshape(NH, HD)[:, _PERM]     # [16, 96]
    bk_p = bk.reshape(NH, HD)[:, _PERM]
    bv_r = bv.reshape(NH, HD)
    bq_g = bq_p.reshape(NCORES, 2, 96).transpose(0, 2, 1)   # [8, 96, 2]
    bk_g = bk_p.reshape(NCORES, 2, 96).transpose(0, 2, 1)
    bv_g = bv_r.reshape(NCORES, 2, 96).transpose(0, 2, 1)

    return {
        "x_s": np.ascontiguousarray(x[0]).astype(BF16N),   # [3072, 1536]
        "wq_n": wq_g, "wk_n": wk_g, "wv_n": wv_g,
        "wo_t": np.ascontiguousarray(wo_g).reshape(NCORES * 97, 2 * DIM),
        "bq_t": np.ascontiguousarray(bq_g, dtype=np.float32
                                     ).reshape(NCORES * 96, 2),
        "bk_t": np.ascontiguousarray(bk_g, dtype=np.float32
                                     ).reshape(NCORES * 96, 2),
        "bv_t": np.ascontiguousarray(bv_g, dtype=np.float32
                                     ).reshape(NCORES * 96, 2),
        "cos_f": np.ascontiguousarray(cos_g).reshape(NCORES * 64, S),
        "sin_f": np.ascontiguousarray(sin_g).reshape(NCORES * 64, S),
    }


def kernel(x, wq, bq, wk, bk, wv, bv, wo, bo, freqs_cos, freqs_sin, h, w):
    fn, in_names, out_names, out_avals, zero_shapes, zero_dtypes = \
        _get_runner()
    gin = make_global_inputs(x, wq, bq, wk, bk, wv, bv, wo, bo,
                             freqs_cos, freqs_sin, h, w)
    args = [gin[nm] for nm in in_names]
    args += [np.zeros(s, d) for s, d in zip(zero_shapes, zero_dtypes)]
    out_arrs = fn(*args)
    og = np.asarray(out_arrs[out_names.index("outS")])   # [8*384, 1536] bf16
    og = og.reshape(NCORES, XSH, DIM)
    out = np.empty((1, S, DIM), np.float32)
    for qb, chunks in RS_CHUNKS.items():
        for (sb0, sb1, oroff) in chunks:
            a = sb0 * 128
            m = (sb1 - sb0) * 128 // NCORES
            for c in range(NCORES):
                out[0, a + c * m: a + (c + 1) * m] = og[c, oroff:oroff + m]
    return out
